# revision 3
# baseline (speedup 1.0000x reference)
"""CRF loss (nn_CRFLoss) on 8 Trainium2 NeuronCores — raw-Bass, 4 chains.

Same math as kernel.py (exp-space forward/backward meet-in-the-middle scan,
fp16 matmuls, fixed kappa pre-scale), but the hot loop is hand-scheduled
raw Bass instead of Tile:

  - exactly one semaphore wait per instruction, attached inline (no
    standalone EVENT_SEMAPHORE churn on the Vector queue),
  - PSUM/SBUF double buffers whose reuse safety is implied by queue order
    (no recycle waits at all),
  - input DMAs issued on otherwise-idle queues (consts on Sync, the four
    emission chunks on Scalar) so the PE/DVE queues only carry real work,
  - optional PE warm-up matmuls + in-loop dummy matmuls to coax the HAM
    clock gate to 2.4 GHz (knobs below).

Per-queue programs (P = 32 props/core, T = 66 tags):
  PE:   [warmup]  LDW+MMf0, LDW+MMb0, LDW+MMf1, ... LDW+MMf63
  DVE:  TTf0, TTb0, TTf1, TTb1, ... TTb62, TTprod
  Sync: consts DMA, output DMA (pre-queued, waits on TTprod)
  Scal: f_exp chunk DMAs (fwd head, bwd head, fwd tail, bwd tail)
"""

import os
import sys

import numpy as np

for _p in ("/opt/trn_rl_repo",):
    if os.path.isdir(_p) and _p not in sys.path:
        sys.path.insert(0, _p)

import concourse.bass as bass
import concourse.mybir as mybir
from concourse import bacc
from concourse.bass_utils import run_bass_kernel_spmd

B, S, V, T = 32, 128, 8, 66
N_CORES = 8
BV = B * V
P = BV // N_CORES          # 32 props per core
NSTEP = S - 1              # 127 transition steps total
MID = 64                   # fwd chain: MMf k=0..63 (steps 1..64)
NBWD = NSTEP - MID         # 63 bwd matmuls (steps 127..65)
NF_DEV = NSTEP - 1         # F blocks on device (t=1..126)
KAPPA = float(np.float32(4.7))

# knobs (test.py may override before first kernel() call)
PROFILE = False
TRACE_TMPDIR = None
LAST_RESULTS = None
NWARM = 0                  # pre-scan warmup matmuls
WARM_N = 256               # their moving free dim
DUM = 0                    # dummy matmuls after each real matmul
DUM_N = 64                 # their moving free dim
FHEAD = 4                  # F blocks per chain shipped inside the consts DMA

_nc_cache = {}

f16 = mybir.dt.float16
f32 = mybir.dt.float32


def _build_bass():
    nc = bacc.Bacc()
    NCONST = 2 * T + 2 * P + 2 * FHEAD * P
    H = P // 2                       # 16 props per sub-chain

    c_in = nc.dram_tensor("consts", [T, NCONST], f16, kind="ExternalInput")
    f_in = nc.dram_tensor("f_exp", [T, NF_DEV * P], f16, kind="ExternalInput")
    prod_out = nc.dram_tensor("prod_out", [T, P], f32, kind="ExternalOutput")

    c_sb = nc.alloc_sbuf_tensor("c_sb", [T, NCONST], f16)
    f_sb = nc.alloc_sbuf_tensor("f_sb", [T, NF_DEV * P], f16)
    # state double-buffers per sub-chain (A = props 0:16, B = 16:32)
    u_sb = [[nc.alloc_sbuf_tensor(f"u{c}{i}", [T, H], f16) for i in range(2)]
            for c in range(2)]
    w_sb = [[nc.alloc_sbuf_tensor(f"w{c}{i}", [T, H], f16) for i in range(2)]
            for c in range(2)]
    prod_sb = nc.alloc_sbuf_tensor("prod_sb", [T, P], f32)

    # 8 PSUM banks: fwd/bwd x sub-chain x double-buffer
    vps = [[nc.alloc_psum_tensor(f"vps{c}{i}", [T, H], f32) for i in range(2)]
           for c in range(2)]
    bps = [[nc.alloc_psum_tensor(f"bps{c}{i}", [T, H], f32) for i in range(2)]
           for c in range(2)]

    E_sb = c_sb[:, 0:T]
    Et_sb = c_sb[:, T:2 * T]
    u0_sb = [c_sb[:, 2 * T + c * H:2 * T + (c + 1) * H] for c in range(2)]
    w0_sb = [c_sb[:, 2 * T + P + c * H:2 * T + P + (c + 1) * H]
             for c in range(2)]
    fh_base = 2 * T + 2 * P            # fwd blocks 0..FHEAD-1
    bh_base = fh_base + FHEAD * P      # bwd blocks NF_DEV-FHEAD..NF_DEV-1

    def f_fwd(k, c):
        if k < FHEAD:
            a = fh_base + k * P + c * H
            return c_sb[:, a:a + H]
        a = k * P + c * H
        return f_sb[:, a:a + H]

    def f_bwd(k, c):
        blk = NF_DEV - 1 - k
        if k < FHEAD:
            pos = blk - (NF_DEV - FHEAD)
            a = bh_base + pos * P + c * H
            return c_sb[:, a:a + H]
        a = blk * P + c * H
        return f_sb[:, a:a + H]

    s_c = nc.alloc_semaphore("s_c")
    s_ff0 = nc.alloc_semaphore("s_ff0")
    s_ff1 = nc.alloc_semaphore("s_ff1")
    s_fb0 = nc.alloc_semaphore("s_fb0")
    s_fb1 = nc.alloc_semaphore("s_fb1")
    s_mf = [nc.alloc_semaphore(f"s_mf{c}") for c in range(2)]
    s_tf = [nc.alloc_semaphore(f"s_tf{c}") for c in range(2)]
    s_mb = [nc.alloc_semaphore(f"s_mb{c}") for c in range(2)]
    s_tb = [nc.alloc_semaphore(f"s_tb{c}") for c in range(2)]
    s_pr = nc.alloc_semaphore("s_pr")
    s_out = nc.alloc_semaphore("s_out")

    # ---- DMA issue ----
    FF0_BLKS = 16
    FB0_BLKS = 16

    def fdma(eng, a, b, sem):
        eng.dma_start(
            out=f_sb[:, a * P:b * P], in_=f_in[:, a * P:b * P]
        ).then_inc(sem, 16)

    nc.sync.dma_start(out=c_sb[:, :], in_=c_in[:, :]).then_inc(s_c, 16)
    fdma(nc.scalar, FHEAD, FF0_BLKS, s_ff0)
    fdma(nc.gpsimd, NF_DEV - FB0_BLKS, NF_DEV - FHEAD, s_fb0)
    fdma(nc.scalar, FF0_BLKS, MID - 1, s_ff1)
    fdma(nc.scalar, MID - 1, NF_DEV - FB0_BLKS, s_fb1)
    nc.sync.dma_start(out=prod_out[:, :], in_=prod_sb[:, :]).wait_op(
        s_pr, 2, "sem-ge").then_inc(s_out, 16)

    def pe_mm(out_ps, wts, mv, load):
        if load:
            nc.tensor.ldweights(wts)
        mm = nc.tensor.matmul(out_ps, wts, mv, start=True, stop=True)
        mm.ins.ldweights = False
        return mm

    nc.tensor.wait_ge(s_c, 16)

    # ---- the scan: 4 sub-chains ----
    for k in range(MID):
        for c in range(2):
            mv = u0_sb[c] if k == 0 else u_sb[c][(k - 1) % 2][:, :]
            mm = pe_mm(vps[c][k % 2][:, :], E_sb, mv, load=(c == 0))
            if k > 0:
                mm.wait_op(s_tf[c], k, "sem-ge")
            mm.then_inc(s_mf[c])
        if k < NBWD:
            for c in range(2):
                mv = w0_sb[c] if k == 0 else w_sb[c][(k - 1) % 2][:, :]
                mm = pe_mm(bps[c][k % 2][:, :], Et_sb, mv, load=(c == 0))
                if k > 0:
                    mm.wait_op(s_tb[c], k, "sem-ge")
                mm.then_inc(s_mb[c])

        if k < MID - 1:
            if k == FHEAD:
                nc.vector.wait_ge(s_ff0, 16)
            elif k == FF0_BLKS:
                nc.vector.wait_ge(s_ff1, 16)
            for c in range(2):
                tt = nc.vector.tensor_mul(
                    u_sb[c][k % 2][:, :], vps[c][k % 2][:, :], f_fwd(k, c))
                tt.wait_op(s_mf[c], k + 1, "sem-ge")
                tt.then_inc(s_tf[c])
        if k < NBWD:
            if k == FHEAD:
                nc.vector.wait_ge(s_fb0, 16)
            elif k == FB0_BLKS:
                nc.vector.wait_ge(s_fb1, 16)
            for c in range(2):
                tt = nc.vector.tensor_mul(
                    w_sb[c][k % 2][:, :], bps[c][k % 2][:, :], f_bwd(k, c))
                tt.wait_op(s_mb[c], k + 1, "sem-ge")
                tt.then_inc(s_tb[c])

    # meet in the middle, per sub-chain
    for c in range(2):
        tt = nc.vector.tensor_mul(
            prod_sb[:, c * H:(c + 1) * H], vps[c][(MID - 1) % 2][:, :],
            w_sb[c][(NBWD - 1) % 2][:, :])
        tt.wait_op(s_mf[c], MID, "sem-ge")
        tt.then_inc(s_pr)

    nc.finalize()
    return nc


def _get_nc():
    key = ("crf-raw4", T, P, NSTEP, MID, FHEAD)
    if key not in _nc_cache:
        _nc_cache[key] = _build_bass()
    return _nc_cache[key]


def kernel(score, transitions, start_transitions, end_transitions,
           v_label, role_label):
    global LAST_RESULTS
    score = np.asarray(score, dtype=np.float32)
    transitions = np.asarray(transitions, dtype=np.float32)
    start_transitions = np.asarray(start_transitions, dtype=np.float32)
    end_transitions = np.asarray(end_transitions, dtype=np.float32)
    vl = np.asarray(v_label).astype(np.int64)
    rl = np.asarray(role_label).astype(np.int64)

    em = np.take_along_axis(score, vl[:, :, None, None], axis=1).reshape(BV, S, T)
    tags = rl.reshape(BV, S)

    # gold path score (host, f64)
    ar = np.arange(BV)
    emit_sc = em[ar[:, None], np.arange(S)[None, :], tags].astype(np.float64).sum(-1)
    tr64 = transitions.astype(np.float64)
    trans_sc = tr64[tags[:, :-1], tags[:, 1:]].sum(-1)
    gold = (start_transitions.astype(np.float64)[tags[:, 0]] + emit_sc
            + trans_sc + end_transitions.astype(np.float64)[tags[:, -1]])

    # device inputs
    E = np.exp(transitions)
    u0 = np.exp(start_transitions[:, None] + em[:, 0, :].T)
    Ft = np.exp(np.transpose(em[:, 1:, :], (2, 1, 0)) - np.float32(KAPPA))
    Ft[:, -1, :] *= np.exp(end_transitions)[:, None]

    nc = _get_nc()
    in_maps = []
    E16 = E.astype(np.float16)
    Et16 = np.ascontiguousarray(E.T).astype(np.float16)
    for m in range(N_CORES):
        sl = slice(m * P, (m + 1) * P)
        F16 = Ft[:, :NF_DEV, sl].astype(np.float16)
        consts = np.concatenate(
            [E16, Et16, u0[:, sl].astype(np.float16),
             Ft[:, -1, sl].astype(np.float16),
             F16[:, :FHEAD].reshape(T, FHEAD * P),
             F16[:, NF_DEV - FHEAD:].reshape(T, FHEAD * P)], axis=1)
        in_maps.append({
            "consts": np.ascontiguousarray(consts),
            "f_exp": np.ascontiguousarray(F16).reshape(T, NF_DEV * P),
        })

    kwargs = {}
    if PROFILE:
        kwargs.update(trace=True, tmpdir=TRACE_TMPDIR)
    res = run_bass_kernel_spmd(nc, in_maps, list(range(N_CORES)), **kwargs)
    LAST_RESULTS = res

    prod = np.concatenate(
        [res.results[m]["prod_out"] for m in range(N_CORES)], axis=1)
    logz = np.log(prod.astype(np.float64).sum(0)) + KAPPA * NSTEP
    nll = (logz - gold).sum() / BV
    return np.float32(nll)
